# revision 20
# baseline (speedup 1.0000x reference)
"""Trainium2 Bass kernel for nn_NeuralTuringMachine (single NTM cell forward).

Math (algebraically reduced from the reference):
  - initial state: h=c=0, rv=0, memory = broadcast(mem_bias), w0 = one-hot(0)
  - gates = x @ W_ih[:, :IN].T + b_ih + b_hh        (rv=0 -> last Wd cols unused;
    f-gate unused since c=0)
  - h = sigmoid(o) * tanh(sigmoid(i) * tanh(g))
  - rp = h @ W_read.T + b_read ; wp = h @ W_write.T + b_write
  - read_w / write_w: content softmax + interpolation gate + circular shift +
    sharpening (memory rows equal mem_bias pre-write, so content addressing is
    a plain matmul against mem_bias)
  - rv = read_w @ mem_bias - erase * ((read_w*write_w) @ mem_bias)
         + add * sum_s(read_w*write_w)        (memory tensor never materialized)
  - out = [h | rv] @ W_out.T + b_out

Sharding: data-parallel over batch (8 cores x 32 rows); weights replicated
per-core in fp16, packed host-side into partition-major contiguous blocks so
each DMA moves multi-KB runs per partition. The LSTM phase runs transposed
(gates.T) so h.T lands directly in the lhsT layout the tail matmuls need and
gate biases are per-partition ACT biases. Two-head addressing runs stacked on
64 partitions (read head rows 0:32, write head rows 32:64). The tail uses only
Exp/Ln activations (sigmoid/tanh rebuilt from exp on DVE) so the ACT engine
loads exactly two function tables, both hidden under DMA waits.
"""

import sys

for _p in ("/opt/trn_rl_repo",):
    if _p not in sys.path:
        sys.path.insert(0, _p)

import numpy as np

import concourse.bass as bass
import concourse.mybir as mybir
from concourse import bacc
from concourse.bass_utils import run_bass_kernel_spmd
from concourse.tile import TileContext, add_dep_helper

# The act-table placement pass picks the FIRST act_func_set containing each
# function; exp and ln then resolve to different tables and every exp<->ln
# alternation costs a ~1.3us table load. Empty those two sets (preserving
# list positions, which are the hardware set ids) so both resolve to
# natural_log_exp_and_others.
_orig_get_act_tables = bacc.get_activation_tables


def _patched_get_act_tables(arch):
    t = dict(_orig_get_act_tables(arch))
    for name in ("exp_and_others", "natural_log"):
        if name in t:
            t[name] = set()
    return t


bacc.get_activation_tables = _patched_get_act_tables

F16 = mybir.dt.float16
F32 = mybir.dt.float32
AF = mybir.ActivationFunctionType
MUL = mybir.AluOpType.mult
ADD = mybir.AluOpType.add

B, IN, H, OUT = 256, 512, 1024, 512
S, Wd, SH = 512, 256, 3
P_READ = Wd + 2 + SH + 1          # 262
NC_ = 8                           # cores
R = B // NC_                      # 32 batch rows per core
HK = H // 128                     # 8 h-chunks
EA = 2 * Wd                       # erase|add width 512

# tail2 packed layout (columns)
T2_MBT = 0                        # [128, 2, 512] fp16  (mem_bias.T, w-major chunks)
T2_MBE = T2_MBT + 2 * S           # [128, 4, 257]       (mem_bias | ones col, s-major)
T2_BR = T2_MBE + 4 * (Wd + 1)     # row0: b_read [262]
T2_BWA = T2_BR + P_READ           # row0: b_write[:262]
T2_BEA = T2_BWA + P_READ          # row0: b_write[262:]
T2_BO = T2_BEA + EA               # row0: b_out [512]
T2_ID = T2_BO + OUT               # rows 0:64: identity 64x64
T2_ONE128 = T2_ID + 64            # all rows: 1.0 (1 col)
T2_ONE64 = T2_ONE128 + 1          # row0: ones [64]
T2_COLS = T2_ONE64 + 64

_cache = {}


def _build():
    nc = bacc.Bacc(trn_type="TRN2")

    xT = nc.dram_tensor("xT", [128, 128], F16, kind="ExternalInput")
    wih = nc.dram_tensor("wih", [8, 128, 1920], F16, kind="ExternalInput")
    cst_a = nc.dram_tensor("cst_a", [128, 97], F16, kind="ExternalInput")
    cst_b = nc.dram_tensor("cst_b", [1, 1676], F16, kind="ExternalInput")
    wrtwwa = nc.dram_tensor("wrtwwa", [128, 4192], F16, kind="ExternalInput")
    mbt_d = nc.dram_tensor("mbt_d", [128, 1024], F16, kind="ExternalInput")
    tail3 = nc.dram_tensor("tail3", [128, 5120], F16, kind="ExternalInput")
    wea_d = nc.dram_tensor("wea_d", [128, 4096], F16, kind="ExternalInput")
    mbe_d = nc.dram_tensor("mbe_d", [128, 1028], F16, kind="ExternalInput")
    y = nc.dram_tensor("y", [R, OUT], F32, kind="ExternalOutput")

    with TileContext(nc) as tc:
        import contextlib

        with contextlib.ExitStack() as ctx:
            singles = ctx.enter_context(tc.tile_pool(name="singles", bufs=1))
            wm_pool = ctx.enter_context(tc.tile_pool(name="wm", bufs=8))
            work = ctx.enter_context(tc.tile_pool(name="work", bufs=2))
            ph = ctx.enter_context(tc.tile_pool(name="ph", bufs=2, space="PSUM"))
            pt = ctx.enter_context(tc.tile_pool(name="pt", bufs=2, space="PSUM"))
            pbig = ctx.enter_context(tc.tile_pool(name="pbig", bufs=1, space="PSUM"))

            # ---------- DMAs (issue order ~ priority) ----------
            t_xt = singles.tile([128, 128], F16, tag="xt")
            _dmas = []

            def _odma(out, in_):
                d = nc.sync.dma_start(out=out, in_=in_)
                if _dmas:
                    add_dep_helper(d.ins, _dmas[-1].ins, sync=False,
                                   reason="input dma issue order")
                _dmas.append(d)
                return d

            t_mbt = singles.tile([128, 1024], F16, tag="mbt")
            t_ca = singles.tile([128, 97], F16, tag="ca")
            t_cb = singles.tile([1, 1676], F16, tag="cb")
            _odma(t_ca[:], cst_a[:])
            _odma(t_mbt[:], mbt_d[:])
            _odma(t_xt[:], xT[:])
            _odma(t_cb[:], cst_b[:])
            t_wm = []
            for m in range(8):
                t = wm_pool.tile([128, 1920], F16, tag="wm")
                _odma(t[:], wih[m])
                t_wm.append(t)
            t_rw = singles.tile([128, 4192], F16, tag="rw")
            _odma(t_rw[:], wrtwwa[:])
            t_we = singles.tile([128, 4096], F16, tag="we")
            _odma(t_we[:], wea_d[:])
            t_mbe = singles.tile([128, 1028], F16, tag="mbe")
            _odma(t_mbe[:], mbe_d[:])
            t_t3 = singles.tile([128, 5120], F16, tag="t3")
            _odma(t_t3[:], tail3[:])

            # views into packed tiles
            xt = t_xt[:].rearrange("p (k r) -> p k r", k=4)
            wrt_v = t_rw[:, 0:2096].rearrange("p (k c) -> p k c", k=8)
            wwa_v = t_rw[:, 2096:4192].rearrange("p (k c) -> p k c", k=8)
            wea_v = t_we[:].rearrange("p (k c) -> p k c", k=8)
            mbt_v = t_mbt[:].rearrange("p (k c) -> p k c", k=2)
            mbe_v = t_mbe[:].rearrange("p (k c) -> p k c", k=4)
            ident = t_ca[0:64, 0:64]
            ones128 = t_ca[:, 64:65]
            ones2 = t_ca[0:2, 65:97]
            onesrow = t_cb[0:1, 0:128]
            ones64 = t_cb[0:1, 0:64]
            brow_r = t_cb[0:1, 128 : 128 + P_READ]
            brow_wa = t_cb[0:1, 390 : 390 + P_READ]
            brow_ea = t_cb[0:1, 652 : 652 + EA]
            brow_o = t_cb[0:1, 1164 : 1164 + OUT]
            woh_v = t_t3[:, 0:4096].rearrange("p (k c) -> p k c", k=8)
            wrv_v = t_t3[:, 4096:5120].rearrange("p (k c) -> p k c", k=2)

            # mn^2 = column sums of mbt^2 — runs at kernel start, before the
            # first sigmoid, so its Ln/Exp use the initial exp table window
            mbt2 = singles.tile([128, 2, S], F16, tag="mbt2")
            nc.gpsimd.tensor_mul(mbt2[:], mbt_v[:, :, :], mbt_v[:, :, :])
            ps_mn = pbig.tile([1, S], F32, tag="pgB")
            for k in range(2):
                nc.tensor.matmul(ps_mn, ones128, mbt2[:, k, :], start=(k == 0), stop=(k == 1))
            mn_l = singles.tile([1, S], F32, tag="mn_l")
            nc.scalar.activation(mn_l, ps_mn, AF.Ln)
            nc.vector.tensor_scalar_mul(mn_l, mn_l, -0.5)
            ivmn = singles.tile([1, S], F16, tag="ivmn")
            nc.scalar.activation(ivmn, mn_l, AF.Exp)

            # ---------- phase 1: gates.T -> h.T ----------
            hT = singles.tile([128, HK + 1, R], F16, tag="hT")
            nc.vector.memset(hT[0:1, HK, :], 1.0)  # ones row (bias matmuls)

            for mp in range(4):
                m0 = 2 * mp
                psg = ph.tile([128, 2, 3 * R], F32, tag="psg")
                for mi in range(2):
                    m = m0 + mi
                    wm_v = t_wm[m][:, 0:1536].rearrange("p (k c) -> p k c", k=4)
                    for j in range(3):
                        for k in range(4):
                            nc.tensor.matmul(
                                psg[:, mi, j * R : (j + 1) * R],
                                wm_v[:, k, 128 * j : 128 * j + 128],
                                xt[:, k, :],
                                start=(k == 0),
                                stop=False,
                            )
                        # bias rows (b_ih, b_hh) via a K=2 rank-update
                        nc.tensor.matmul(
                            psg[:, mi, j * R : (j + 1) * R],
                            t_wm[m][0:2, 1536 + 128 * j : 1536 + 128 * j + 128],
                            ones2,
                            start=False,
                            stop=True,
                        )
                # [128, 2, R] activations: both m of the pair in one ACT op
                si = work.tile([128, 2, R], F32, tag="si")
                tg = work.tile([128, 2, R], F32, tag="tg")
                nc.scalar.activation(si, psg[:, :, 0:R], AF.Sigmoid)
                nc.scalar.activation(tg, psg[:, :, R : 2 * R], AF.Tanh)
                cc = work.tile([128, 2, R], F32, tag="cc")
                nc.vector.tensor_mul(cc, si, tg)
                tc_ = work.tile([128, 2, R], F32, tag="tc_")
                nc.scalar.activation(tc_, cc, AF.Tanh)
                so = work.tile([128, 2, R], F32, tag="so")
                nc.scalar.activation(so, psg[:, :, 2 * R : 3 * R], AF.Sigmoid)
                nc.vector.tensor_mul(hT[:, m0 : m0 + 2, :], so, tc_)

            # prefetch the exp/ln ACT table during the post-phase-1 ACT idle
            dummy = work.tile([1, 1], F32, tag="dummy")
            nc.scalar.activation(dummy, ones128[0:1, :], AF.Exp)

            # ---------- rp/wp matmuls (read head rows 0:32, write head rows 32:64) ----------
            psA1 = pbig.tile([R, P_READ], F32, tag="pgA")
            psA2 = pbig.tile([64, P_READ], F32, tag="pgA2")
            for k in range(HK + 1):
                lA = hT[:, k, :] if k < HK else hT[0:1, HK, :]
                rA = wrt_v[:, k, :] if k < HK else brow_r
                rB = wwa_v[:, k, :] if k < HK else brow_wa
                nc.tensor.matmul(psA1, lA, rA, start=(k == 0), stop=(k == HK))
                nc.tensor.matmul(psA2[R:64, :], lA, rB, start=(k == 0), stop=(k == HK))

            # broadcast inv_mn over all partitions; pre-normalize mbt columns
            psB = pbig.tile([128, S], F32, tag="pgB")
            nc.tensor.matmul(psB, onesrow, ivmn[:], start=True, stop=True)
            mbtN = singles.tile([128, 2, S], F16, tag="mbtN")
            nc.vector.tensor_mul(mbtN[:, 0, :], mbt_v[:, 0, :], psB)
            nc.vector.tensor_mul(mbtN[:, 1, :], mbt_v[:, 1, :], psB)


            # ---------- addressing (all-exp tail) ----------
            kst = work.tile([64, Wd], F16, tag="kst")
            nc.vector.tensor_copy(kst[0:R, :], psA1[:, 0:Wd])
            nc.vector.tensor_copy(kst[R:64, :], psA2[R:64, 0:Wd])
            scal = work.tile([64, 6], F32, tag="scal")
            nc.vector.tensor_copy(scal[0:R, :], psA1[:, Wd : Wd + 6])
            nc.vector.tensor_copy(scal[R:64, :], psA2[R:64, Wd : Wd + 6])


            # per-head scalars, all from one Exp over scal
            es = work.tile([64, 6], F32, tag="es")
            nc.scalar.activation(es, scal, AF.Exp)
            lnin = work.tile([64, 2], F32, tag="lnin")
            nc.vector.tensor_scalar_add(lnin, es[:, 0:6:5], 1.0)
            bg_l = work.tile([64, 2], F32, tag="bg_l")
            nc.scalar.activation(bg_l, lnin, AF.Ln)            # softplus(beta), softplus(gamma)
            gam1 = work.tile([64, 1], F32, tag="gam1")
            nc.vector.tensor_scalar_add(gam1, bg_l[:, 1:2], 1.0)
            rec_in = work.tile([64, 2], F32, tag="rec_in")
            nc.vector.tensor_scalar_add(rec_in[:, 0:1], es[:, 1:2], 1.0)
            nc.vector.reduce_sum(rec_in[:, 1:2], es[:, 2:5], axis=mybir.AxisListType.X)
            rec = work.tile([64, 2], F32, tag="rec")
            nc.vector.reciprocal(rec, rec_in)
            og = rec[:, 0:1]                                   # og = 1 - gate
            gate = work.tile([64, 1], F32, tag="gate")
            nc.vector.tensor_mul(gate, es[:, 1:2], og)         # sigmoid(gate)
            shn = work.tile([64, SH], F32, tag="shn")
            nc.vector.tensor_scalar_mul(shn, es[:, 2:5], rec[:, 1:2])

            # beta / ||key||
            kn2 = work.tile([64, Wd], F32, tag="kn2")
            kn2s = work.tile([64, 1], F32, tag="kn2s")
            nc.scalar.activation(kn2, kst, AF.Square, accum_out=kn2s)
            kn_l = work.tile([64, 1], F32, tag="kn_l")
            nc.scalar.activation(kn_l, kn2s, AF.Ln)
            nc.vector.tensor_scalar_mul(kn_l, kn_l, -0.5)
            ikn = work.tile([64, 1], F32, tag="ikn")
            nc.scalar.activation(ikn, kn_l, AF.Exp)
            bikn = work.tile([64, 1], F32, tag="bikn")
            nc.vector.tensor_mul(bikn, bg_l[:, 0:1], ikn)

            # key.T then content scores
            kT = work.tile([128, 2, 64], F16, tag="kT")
            for j in range(2):
                tp = pt.tile([128, 64], F16, tag="tp")
                nc.tensor.transpose(tp, kst[:, 128 * j : 128 * (j + 1)], ident)
                nc.vector.tensor_copy(kT[:, j, :], tp)
            psN = pbig.tile([64, S], F32, tag="pgB")
            for j in range(2):
                nc.tensor.matmul(psN, kT[:, j, :], mbtN[:, j, :], start=(j == 0), stop=(j == 1))

            # erase|add pre-activations (consumed late, scheduled off-chain)
            psC = pbig.tile([R, EA], F32, tag="pgC")
            for k in range(HK + 1):
                lA = hT[:, k, :] if k < HK else hT[0:1, HK, :]
                rC = wea_v[:, k, :] if k < HK else brow_ea
                nc.tensor.matmul(psC, lA, rC, start=(k == 0), stop=(k == HK))

            ex = work.tile([64, S], F32, tag="ex")
            exs = work.tile([64, 1], F32, tag="exs")
            nc.scalar.activation(ex, psN, AF.Exp, scale=bikn, accum_out=exs)
            exr = work.tile([64, 1], F32, tag="exr")
            nc.vector.reciprocal_approx_fast(exr, exs)
            gcw = work.tile([64, 1], F32, tag="gcw")
            nc.vector.tensor_mul(gcw, gate, exr)
            gw = work.tile([64, S], F32, tag="gw")
            nc.vector.tensor_scalar_mul(gw, ex, gcw)              # gate * softmax
            nc.vector.tensor_add(gw[:, 0:1], gw[:, 0:1], og)      # + (1-gate) * w0

            # erase/add exps (ACT-side; DVE part comes after the shift)
            ee = work.tile([R, EA], F32, tag="ee")
            nc.scalar.activation(ee[:, 0:Wd], psC[:, 0:Wd], AF.Exp)
            nc.scalar.activation(ee[:, Wd:EA], psC[:, Wd:EA], AF.Exp, scale=2.0)

            # circular shift (mul+add pairs fused via scalar_tensor_tensor)
            sw = work.tile([64, S], F32, tag="sw")
            nc.vector.tensor_scalar_mul(sw, gw, shn[:, 1:2])
            nc.vector.scalar_tensor_tensor(
                sw[:, 1:S], gw[:, 0 : S - 1], shn[:, 0:1], sw[:, 1:S], MUL, ADD)
            nc.vector.scalar_tensor_tensor(
                sw[:, 0:1], gw[:, S - 1 : S], shn[:, 0:1], sw[:, 0:1], MUL, ADD)
            nc.vector.scalar_tensor_tensor(
                sw[:, 0 : S - 1], gw[:, 1:S], shn[:, 2:3], sw[:, 0 : S - 1], MUL, ADD)
            nc.vector.scalar_tensor_tensor(
                sw[:, S - 1 : S], gw[:, 0:1], shn[:, 2:3], sw[:, S - 1 : S], MUL, ADD)

            # erase/add DVE part (fills DVE while ACT does Ln/Exp sharpening)
            ee1 = work.tile([R, EA], F32, tag="ee1")
            nc.vector.tensor_scalar_add(ee1, ee, 1.0)
            rr = work.tile([R, EA], F32, tag="rr")
            nc.vector.reciprocal_approx_fast(rr, ee1)
            er_ = work.tile([R, Wd], F32, tag="er_")
            nc.vector.tensor_mul(er_, ee[:, 0:Wd], rr[:, 0:Wd])      # sigmoid
            eAm = work.tile([R, Wd], F32, tag="eAm")
            nc.vector.tensor_scalar_add(eAm, ee[:, Wd:EA], -1.0)
            ad_ = work.tile([R, Wd], F32, tag="ad_")
            nc.vector.tensor_mul(ad_, eAm, rr[:, Wd:EA])             # tanh

            # sharpening + normalize
            lnw = work.tile([64, S], F32, tag="lnw")
            nc.scalar.activation(lnw, sw, AF.Ln)
            sp = work.tile([64, S], F32, tag="sp")
            sps = work.tile([64, 1], F32, tag="sps")
            nc.scalar.activation(sp, lnw, AF.Exp, scale=gam1, accum_out=sps)
            nc.vector.tensor_scalar_add(sps, sps, 1e-6)
            spr = work.tile([64, 1], F32, tag="spr")
            nc.vector.reciprocal_approx_fast(spr, sps)
            st16 = work.tile([64, S], F16, tag="st16")
            nc.vector.tensor_scalar_mul(st16, sp, spr)              # stacked read_w/write_w

            # out h-part early; accumulation group stays open for the rv part
            psOut = ph.tile([R, OUT], F32, tag="psg")
            for k in range(HK):
                nc.tensor.matmul(psOut, hT[:, k, :], woh_v[:, k, :],
                                 start=(k == 0), stop=False, skip_group_check=True)

            # transpose stacked weights; head product in transposed space;
            # t1 = read_w @ [mem_bias|1] ; t2|srw = (read_w*write_w) @ [mem_bias|1]
            stT = work.tile([128, 4, 64], F16, tag="stT")
            rwT = work.tile([128, 4, R], F16, tag="rwT")
            psT1 = pbig.tile([R, Wd + 1], F32, tag="pgA")
            psT2 = pbig.tile([R, Wd + 1], F32, tag="pgB")
            for j in range(4):
                tp = pt.tile([128, 64], F16, tag="tp")
                nc.tensor.transpose(tp, st16[:, 128 * j : 128 * (j + 1)], ident)
                nc.vector.tensor_copy(stT[:, j, :], tp)
                nc.vector.tensor_mul(rwT[:, j, :], stT[:, j, 0:R], stT[:, j, R:64])
                nc.tensor.matmul(psT1, stT[:, j, 0:R], mbe_v[:, j, :], start=(j == 0), stop=(j == 3))
                nc.tensor.matmul(psT2, rwT[:, j, :], mbe_v[:, j, :], start=(j == 0), stop=(j == 3))


            # rv = t1 - erase*t2 + add*srw
            x1 = work.tile([R, Wd], F32, tag="x1")
            nc.vector.tensor_mul(x1, er_, psT2[:, 0:Wd])
            x2 = work.tile([R, Wd], F32, tag="x2")
            nc.vector.tensor_sub(x2, psT1[:, 0:Wd], x1)
            x3 = work.tile([R, Wd], F32, tag="x3")
            nc.vector.tensor_scalar_mul(x3, ad_, psT2[:, Wd : Wd + 1])
            rv16 = work.tile([R, Wd], F16, tag="rv16")
            nc.vector.tensor_add(rv16, x2, x3)

            # rv.T (+ones row) closes the out accumulation
            rvT = singles.tile([128, 3, R], F16, tag="rvT")
            nc.vector.memset(rvT[0:1, 2, :], 1.0)
            for j in range(2):
                tp = pt.tile([128, R], F16, tag="tp")
                nc.tensor.transpose(tp, rv16[:, 128 * j : 128 * (j + 1)], ident[0:R, 0:R])
                nc.vector.tensor_copy(rvT[:, j, :], tp)
            nc.tensor.matmul(psOut, rvT[:, 0, :], wrv_v[:, 0, :], start=False, stop=False,
                             skip_group_check=True)
            nc.tensor.matmul(psOut, rvT[:, 1, :], wrv_v[:, 1, :], start=False, stop=False,
                             skip_group_check=True)
            nc.tensor.matmul(psOut, rvT[0:1, 2, :], brow_o, start=False, stop=True,
                             skip_group_check=True)

            yout = work.tile([R, OUT], F32, tag="yout")
            nc.vector.tensor_copy(yout, psOut)
            nc.sync.dma_start(out=y[:], in_=yout[:])

    nc.finalize()
    return nc


def _kp(a, kc):
    """[kc*128, c] -> [128, kc*c] partition-major packed."""
    c = a.shape[1]
    return a.reshape(kc, 128, c).transpose(1, 0, 2).reshape(128, kc * c)


def _prep(inputs):
    f16 = np.float16
    x = np.asarray(inputs["x"], np.float32)
    W_ih = np.asarray(inputs["W_ih"], np.float32)
    b_ih = np.asarray(inputs["b_ih"], np.float32)
    b_hh = np.asarray(inputs["b_hh"], np.float32)
    W_read = np.asarray(inputs["W_read"], np.float32)
    b_read = np.asarray(inputs["b_read"], np.float32)
    W_write = np.asarray(inputs["W_write"], np.float32)
    b_write = np.asarray(inputs["b_write"], np.float32)
    W_out = np.asarray(inputs["W_out"], np.float32)
    b_out = np.asarray(inputs["b_out"], np.float32)
    mem_bias = np.asarray(inputs["mem_bias"], np.float32)

    i0, g0, o0 = 0, 2 * H, 3 * H
    wih = np.zeros((8, 128, 1920), f16)
    for m in range(8):
        sl = [slice(b0 + 128 * m, b0 + 128 * m + 128) for b0 in (i0, g0, o0)]
        blk = np.concatenate([W_ih[s, :IN] for s in sl], axis=0)  # [384, 512]
        wih[m, :, 0:1536] = _kp(blk.T.astype(f16), 4)
        for j, s in enumerate(sl):
            wih[m, 0, 1536 + 128 * j : 1536 + 128 * (j + 1)] = b_ih[s].astype(f16)
            wih[m, 1, 1536 + 128 * j : 1536 + 128 * (j + 1)] = b_hh[s].astype(f16)

    wrtwwa = np.concatenate([
        _kp(W_read.T.astype(f16), 8),               # [128, 2096]
        _kp(W_write[:P_READ, :].T.astype(f16), 8),  # [128, 2096]
    ], axis=1)
    wea_d = _kp(W_write[P_READ:, :].T.astype(f16), 8)   # [128, 4096]
    mbt_d = _kp(mem_bias.T.astype(f16), 2)              # [128, 1024]
    mbe = np.concatenate([mem_bias, np.ones((S, 1), np.float32)], axis=1).astype(f16)
    mbe_d = _kp(mbe, 4)                                  # [128, 1028]

    cst_a = np.zeros((128, 97), f16)
    cst_a[0:64, 0:64] = np.eye(64, dtype=f16)
    cst_a[:, 64] = 1.0
    cst_a[0:2, 65:97] = 1.0
    cst_b = np.zeros((1, 1676), f16)
    cst_b[0, 0:128] = 1.0
    cst_b[0, 128 : 128 + P_READ] = b_read.astype(f16)
    cst_b[0, 390 : 390 + P_READ] = b_write[:P_READ].astype(f16)
    cst_b[0, 652 : 652 + EA] = b_write[P_READ:].astype(f16)
    cst_b[0, 1164 : 1164 + OUT] = b_out.astype(f16)

    tail3 = np.concatenate([
        _kp(W_out[:, :H].T.astype(f16), 8),         # [128, 4096]
        _kp(W_out[:, H:].T.astype(f16), 2),         # [128, 1024]
    ], axis=1)

    shared = dict(wih=wih, cst_a=cst_a, cst_b=cst_b, wrtwwa=wrtwwa,
                  mbt_d=mbt_d, wea_d=wea_d, mbe_d=mbe_d, tail3=tail3)
    in_maps = []
    for c in range(NC_):
        m = dict(shared)
        m["xT"] = _kp(np.ascontiguousarray(x[c * R : (c + 1) * R].T).astype(f16), 4)
        in_maps.append(m)
    return in_maps


def kernel(**inputs) -> np.ndarray:
    if "nc" not in _cache:
        _cache["nc"] = _build()
    nc = _cache["nc"]
    in_maps = _prep(inputs)
    res = run_bass_kernel_spmd(nc, in_maps, core_ids=list(range(NC_)))
    return np.concatenate([res.results[c]["y"] for c in range(NC_)], axis=0)


# revision 21
# speedup vs baseline: 1.0683x; 1.0683x over previous
"""Trainium2 Bass kernel for nn_NeuralTuringMachine (single NTM cell forward).

Math (algebraically reduced from the reference):
  - initial state: h=c=0, rv=0, memory = broadcast(mem_bias), w0 = one-hot(0)
  - gates = x @ W_ih[:, :IN].T + b_ih + b_hh        (rv=0 -> last Wd cols unused;
    f-gate unused since c=0)
  - h = sigmoid(o) * tanh(sigmoid(i) * tanh(g))
  - rp = h @ W_read.T + b_read ; wp = h @ W_write.T + b_write
  - read_w / write_w: content softmax + interpolation gate + circular shift +
    sharpening (memory rows equal mem_bias pre-write, so content addressing is
    a plain matmul against mem_bias)
  - rv = read_w @ mem_bias - erase * ((read_w*write_w) @ mem_bias)
         + add * sum_s(read_w*write_w)        (memory tensor never materialized)
  - out = [h | rv] @ W_out.T + b_out

Sharding: data-parallel over batch (8 cores x 32 rows); weights replicated
per-core in fp16, packed host-side into partition-major contiguous blocks so
each DMA moves multi-KB runs per partition. The LSTM phase runs transposed
(gates.T) so h.T lands directly in the lhsT layout the tail matmuls need and
gate biases are per-partition ACT biases. Two-head addressing runs stacked on
64 partitions (read head rows 0:32, write head rows 32:64). The tail uses only
Exp/Ln activations (sigmoid/tanh rebuilt from exp on DVE) so the ACT engine
loads exactly two function tables, both hidden under DMA waits.
"""

import sys

for _p in ("/opt/trn_rl_repo",):
    if _p not in sys.path:
        sys.path.insert(0, _p)

import numpy as np

import concourse.bass as bass
import concourse.mybir as mybir
from concourse import bacc
from concourse.bass_utils import run_bass_kernel_spmd
from concourse.tile import TileContext, add_dep_helper

# The act-table placement pass picks the FIRST act_func_set containing each
# function; exp and ln then resolve to different tables and every exp<->ln
# alternation costs a ~1.3us table load. Empty those two sets (preserving
# list positions, which are the hardware set ids) so both resolve to
# natural_log_exp_and_others.
_orig_get_act_tables = bacc.get_activation_tables


def _patched_get_act_tables(arch):
    t = dict(_orig_get_act_tables(arch))
    for name in ("exp_and_others", "natural_log"):
        if name in t:
            t[name] = set()
    return t


bacc.get_activation_tables = _patched_get_act_tables

F16 = mybir.dt.float16
F32 = mybir.dt.float32
AF = mybir.ActivationFunctionType
MUL = mybir.AluOpType.mult
ADD = mybir.AluOpType.add

B, IN, H, OUT = 256, 512, 1024, 512
S, Wd, SH = 512, 256, 3
P_READ = Wd + 2 + SH + 1          # 262
NC_ = 8                           # cores
R = B // NC_                      # 32 batch rows per core
HK = H // 128                     # 8 h-chunks
EA = 2 * Wd                       # erase|add width 512

# tail2 packed layout (columns)
T2_MBT = 0                        # [128, 2, 512] fp16  (mem_bias.T, w-major chunks)
T2_MBE = T2_MBT + 2 * S           # [128, 4, 257]       (mem_bias | ones col, s-major)
T2_BR = T2_MBE + 4 * (Wd + 1)     # row0: b_read [262]
T2_BWA = T2_BR + P_READ           # row0: b_write[:262]
T2_BEA = T2_BWA + P_READ          # row0: b_write[262:]
T2_BO = T2_BEA + EA               # row0: b_out [512]
T2_ID = T2_BO + OUT               # rows 0:64: identity 64x64
T2_ONE128 = T2_ID + 64            # all rows: 1.0 (1 col)
T2_ONE64 = T2_ONE128 + 1          # row0: ones [64]
T2_COLS = T2_ONE64 + 64

_cache = {}


def _build():
    nc = bacc.Bacc(trn_type="TRN2")

    xT = nc.dram_tensor("xT", [128, 128], F16, kind="ExternalInput")
    wih = nc.dram_tensor("wih", [8, 128, 1536], F16, kind="ExternalInput")
    wbias = nc.dram_tensor("wbias", [2, 3072], F16, kind="ExternalInput")
    cst_a = nc.dram_tensor("cst_a", [128, 97], F16, kind="ExternalInput")
    cst_b = nc.dram_tensor("cst_b", [1, 1676], F16, kind="ExternalInput")
    wrtwwa = nc.dram_tensor("wrtwwa", [128, 4192], F16, kind="ExternalInput")
    mbt_d = nc.dram_tensor("mbt_d", [128, 1024], F16, kind="ExternalInput")
    tail3 = nc.dram_tensor("tail3", [128, 5120], F16, kind="ExternalInput")
    wea_d = nc.dram_tensor("wea_d", [128, 4096], F16, kind="ExternalInput")
    mbe_d = nc.dram_tensor("mbe_d", [128, 1028], F16, kind="ExternalInput")
    y = nc.dram_tensor("y", [R, OUT], F32, kind="ExternalOutput")

    with TileContext(nc) as tc:
        import contextlib

        with contextlib.ExitStack() as ctx:
            singles = ctx.enter_context(tc.tile_pool(name="singles", bufs=1))
            wm_pool = ctx.enter_context(tc.tile_pool(name="wm", bufs=8))
            work = ctx.enter_context(tc.tile_pool(name="work", bufs=2))
            ph = ctx.enter_context(tc.tile_pool(name="ph", bufs=2, space="PSUM"))
            pt = ctx.enter_context(tc.tile_pool(name="pt", bufs=2, space="PSUM"))
            pbig = ctx.enter_context(tc.tile_pool(name="pbig", bufs=1, space="PSUM"))

            # ---------- DMAs (issue order ~ priority) ----------
            t_xt = singles.tile([128, 128], F16, tag="xt")
            _dmas = []

            def _odma(out, in_):
                d = nc.sync.dma_start(out=out, in_=in_)
                if _dmas:
                    add_dep_helper(d.ins, _dmas[-1].ins, sync=False,
                                   reason="input dma issue order")
                _dmas.append(d)
                return d

            t_mbt = singles.tile([128, 1024], F16, tag="mbt")
            t_ca = singles.tile([128, 97], F16, tag="ca")
            t_cb = singles.tile([1, 1676], F16, tag="cb")
            _odma(t_ca[:], cst_a[:])
            _odma(t_mbt[:], mbt_d[:])
            _odma(t_xt[:], xT[:])
            _odma(t_cb[:], cst_b[:])
            t_wb = singles.tile([2, 3072], F16, tag="wb")
            _odma(t_wb[:], wbias[:])
            t_wm = []
            for m in range(8):
                t = wm_pool.tile([128, 1536], F16, tag="wm")
                _odma(t[:], wih[m])
                t_wm.append(t)
            t_rw = singles.tile([128, 4192], F16, tag="rw")
            _odma(t_rw[:], wrtwwa[:])
            t_we = singles.tile([128, 4096], F16, tag="we")
            _odma(t_we[:], wea_d[:])
            t_mbe = singles.tile([128, 1028], F16, tag="mbe")
            _odma(t_mbe[:], mbe_d[:])
            t_t3 = singles.tile([128, 5120], F16, tag="t3")
            _odma(t_t3[:], tail3[:])

            # views into packed tiles
            xt = t_xt[:].rearrange("p (k r) -> p k r", k=4)
            wrt_v = t_rw[:, 0:2096].rearrange("p (k c) -> p k c", k=8)
            wwa_v = t_rw[:, 2096:4192].rearrange("p (k c) -> p k c", k=8)
            wea_v = t_we[:].rearrange("p (k c) -> p k c", k=8)
            mbt_v = t_mbt[:].rearrange("p (k c) -> p k c", k=2)
            mbe_v = t_mbe[:].rearrange("p (k c) -> p k c", k=4)
            ident = t_ca[0:64, 0:64]
            ones128 = t_ca[:, 64:65]
            ones2 = t_ca[0:2, 65:97]
            onesrow = t_cb[0:1, 0:128]
            ones64 = t_cb[0:1, 0:64]
            brow_r = t_cb[0:1, 128 : 128 + P_READ]
            brow_wa = t_cb[0:1, 390 : 390 + P_READ]
            brow_ea = t_cb[0:1, 652 : 652 + EA]
            brow_o = t_cb[0:1, 1164 : 1164 + OUT]
            woh_v = t_t3[:, 0:4096].rearrange("p (k c) -> p k c", k=8)
            wrv_v = t_t3[:, 4096:5120].rearrange("p (k c) -> p k c", k=2)

            # mn^2 = column sums of mbt^2 — runs at kernel start, before the
            # first sigmoid, so its Ln/Exp use the initial exp table window
            mbt2 = singles.tile([128, 2, S], F16, tag="mbt2")
            nc.gpsimd.tensor_mul(mbt2[:], mbt_v[:, :, :], mbt_v[:, :, :])
            ps_mn = pbig.tile([1, S], F32, tag="pgB")
            for k in range(2):
                nc.tensor.matmul(ps_mn, ones128, mbt2[:, k, :], start=(k == 0), stop=(k == 1))
            mn_l = singles.tile([1, S], F32, tag="mn_l")
            nc.scalar.activation(mn_l, ps_mn, AF.Ln)
            nc.vector.tensor_scalar_mul(mn_l, mn_l, -0.5)
            ivmn = singles.tile([1, S], F16, tag="ivmn")
            nc.scalar.activation(ivmn, mn_l, AF.Exp)

            # ---------- phase 1: gates.T -> h.T ----------
            hT = singles.tile([128, HK + 1, R], F16, tag="hT")
            nc.vector.memset(hT[0:1, HK, :], 1.0)  # ones row (bias matmuls)

            for mp in range(4):
                m0 = 2 * mp
                psg = ph.tile([128, 2, 3 * R], F32, tag="psg")
                for mi in range(2):
                    m = m0 + mi
                    wm_v = t_wm[m][:].rearrange("p (k c) -> p k c", k=4)
                    for j in range(3):
                        for k in range(4):
                            nc.tensor.matmul(
                                psg[:, mi, j * R : (j + 1) * R],
                                wm_v[:, k, 128 * j : 128 * j + 128],
                                xt[:, k, :],
                                start=(k == 0),
                                stop=False,
                            )
                        # bias rows (b_ih, b_hh) via a K=2 rank-update
                        nc.tensor.matmul(
                            psg[:, mi, j * R : (j + 1) * R],
                            t_wb[:, (m * 3 + j) * 128 : (m * 3 + j) * 128 + 128],
                            ones2,
                            start=False,
                            stop=True,
                        )
                # [128, 2, R] activations: both m of the pair in one ACT op
                si = work.tile([128, 2, R], F32, tag="si")
                tg = work.tile([128, 2, R], F32, tag="tg")
                nc.scalar.activation(si, psg[:, :, 0:R], AF.Sigmoid)
                nc.scalar.activation(tg, psg[:, :, R : 2 * R], AF.Tanh)
                cc = work.tile([128, 2, R], F32, tag="cc")
                nc.vector.tensor_mul(cc, si, tg)
                tc_ = work.tile([128, 2, R], F32, tag="tc_")
                nc.scalar.activation(tc_, cc, AF.Tanh)
                so = work.tile([128, 2, R], F32, tag="so")
                nc.scalar.activation(so, psg[:, :, 2 * R : 3 * R], AF.Sigmoid)
                nc.vector.tensor_mul(hT[:, m0 : m0 + 2, :], so, tc_)

            # ---------- rp/wp matmuls (read head rows 0:32, write head rows 32:64) ----------
            psA1 = pbig.tile([R, P_READ], F32, tag="pgA")
            psA2 = pbig.tile([64, P_READ], F32, tag="pgA2")
            for k in range(HK + 1):
                lA = hT[:, k, :] if k < HK else hT[0:1, HK, :]
                rA = wrt_v[:, k, :] if k < HK else brow_r
                rB = wwa_v[:, k, :] if k < HK else brow_wa
                nc.tensor.matmul(psA1, lA, rA, start=(k == 0), stop=(k == HK))
                nc.tensor.matmul(psA2[R:64, :], lA, rB, start=(k == 0), stop=(k == HK))

            # broadcast inv_mn over all partitions; pre-normalize mbt columns
            psB = pbig.tile([128, S], F32, tag="pgB")
            nc.tensor.matmul(psB, onesrow, ivmn[:], start=True, stop=True)
            mbtN = singles.tile([128, 2, S], F16, tag="mbtN")
            nc.vector.tensor_mul(mbtN[:, 0, :], mbt_v[:, 0, :], psB)
            nc.vector.tensor_mul(mbtN[:, 1, :], mbt_v[:, 1, :], psB)


            # ---------- addressing (all-exp tail) ----------
            kst = work.tile([64, Wd], F16, tag="kst")
            nc.vector.tensor_copy(kst[0:R, :], psA1[:, 0:Wd])
            nc.vector.tensor_copy(kst[R:64, :], psA2[R:64, 0:Wd])
            scal = work.tile([64, 6], F32, tag="scal")
            nc.vector.tensor_copy(scal[0:R, :], psA1[:, Wd : Wd + 6])
            nc.vector.tensor_copy(scal[R:64, :], psA2[R:64, Wd : Wd + 6])


            # per-head scalars, all from one Exp over scal
            es = work.tile([64, 6], F32, tag="es")
            nc.scalar.activation(es, scal, AF.Exp)
            lnin = work.tile([64, 2], F32, tag="lnin")
            nc.vector.tensor_scalar_add(lnin, es[:, 0:6:5], 1.0)
            bg_l = work.tile([64, 2], F32, tag="bg_l")
            nc.scalar.activation(bg_l, lnin, AF.Ln)            # softplus(beta), softplus(gamma)
            gam1 = work.tile([64, 1], F32, tag="gam1")
            nc.vector.tensor_scalar_add(gam1, bg_l[:, 1:2], 1.0)
            rec_in = work.tile([64, 2], F32, tag="rec_in")
            nc.vector.tensor_scalar_add(rec_in[:, 0:1], es[:, 1:2], 1.0)
            nc.vector.reduce_sum(rec_in[:, 1:2], es[:, 2:5], axis=mybir.AxisListType.X)
            rec = work.tile([64, 2], F32, tag="rec")
            nc.vector.reciprocal(rec, rec_in)
            og = rec[:, 0:1]                                   # og = 1 - gate
            gate = work.tile([64, 1], F32, tag="gate")
            nc.vector.tensor_mul(gate, es[:, 1:2], og)         # sigmoid(gate)
            shn = work.tile([64, SH], F32, tag="shn")
            nc.vector.tensor_scalar_mul(shn, es[:, 2:5], rec[:, 1:2])

            # beta / ||key||
            kn2 = work.tile([64, Wd], F32, tag="kn2")
            nc.vector.tensor_mul(kn2, kst, kst)
            kn2s = work.tile([64, 1], F32, tag="kn2s")
            nc.vector.reduce_sum(kn2s, kn2, axis=mybir.AxisListType.X)
            kn_l = work.tile([64, 1], F32, tag="kn_l")
            nc.scalar.activation(kn_l, kn2s, AF.Ln)
            nc.vector.tensor_scalar_mul(kn_l, kn_l, -0.5)
            ikn = work.tile([64, 1], F32, tag="ikn")
            nc.scalar.activation(ikn, kn_l, AF.Exp)
            bikn = work.tile([64, 1], F32, tag="bikn")
            nc.vector.tensor_mul(bikn, bg_l[:, 0:1], ikn)

            # key.T then content scores
            kT = work.tile([128, 2, 64], F16, tag="kT")
            for j in range(2):
                tp = pt.tile([128, 64], F16, tag="tp")
                nc.tensor.transpose(tp, kst[:, 128 * j : 128 * (j + 1)], ident)
                nc.vector.tensor_copy(kT[:, j, :], tp)
            psN = pbig.tile([64, S], F32, tag="pgB")
            for j in range(2):
                nc.tensor.matmul(psN, kT[:, j, :], mbtN[:, j, :], start=(j == 0), stop=(j == 1))

            # erase|add pre-activations (consumed late, scheduled off-chain)
            psC = pbig.tile([R, EA], F32, tag="pgC")
            for k in range(HK + 1):
                lA = hT[:, k, :] if k < HK else hT[0:1, HK, :]
                rC = wea_v[:, k, :] if k < HK else brow_ea
                nc.tensor.matmul(psC, lA, rC, start=(k == 0), stop=(k == HK))

            ex = work.tile([64, S], F32, tag="ex")
            exs = work.tile([64, 1], F32, tag="exs")
            nc.scalar.activation(ex, psN, AF.Exp, scale=bikn, accum_out=exs)
            exr = work.tile([64, 1], F32, tag="exr")
            nc.vector.reciprocal_approx_fast(exr, exs)
            gcw = work.tile([64, 1], F32, tag="gcw")
            nc.vector.tensor_mul(gcw, gate, exr)
            gw = work.tile([64, S], F32, tag="gw")
            nc.vector.tensor_scalar_mul(gw, ex, gcw)              # gate * softmax
            nc.vector.tensor_add(gw[:, 0:1], gw[:, 0:1], og)      # + (1-gate) * w0

            # erase/add exps (ACT-side; DVE part comes after the shift)
            ee = work.tile([R, EA], F32, tag="ee")
            nc.scalar.activation(ee[:, 0:Wd], psC[:, 0:Wd], AF.Exp)
            nc.scalar.activation(ee[:, Wd:EA], psC[:, Wd:EA], AF.Exp, scale=2.0)

            # circular shift (mul+add pairs fused via scalar_tensor_tensor)
            sw = work.tile([64, S], F32, tag="sw")
            nc.vector.tensor_scalar_mul(sw, gw, shn[:, 1:2])
            nc.vector.scalar_tensor_tensor(
                sw[:, 1:S], gw[:, 0 : S - 1], shn[:, 0:1], sw[:, 1:S], MUL, ADD)
            nc.vector.scalar_tensor_tensor(
                sw[:, 0:1], gw[:, S - 1 : S], shn[:, 0:1], sw[:, 0:1], MUL, ADD)
            nc.vector.scalar_tensor_tensor(
                sw[:, 0 : S - 1], gw[:, 1:S], shn[:, 2:3], sw[:, 0 : S - 1], MUL, ADD)
            nc.vector.scalar_tensor_tensor(
                sw[:, S - 1 : S], gw[:, 0:1], shn[:, 2:3], sw[:, S - 1 : S], MUL, ADD)

            # erase/add DVE part (fills DVE while ACT does Ln/Exp sharpening)
            ee1 = work.tile([R, EA], F32, tag="ee1")
            nc.vector.tensor_scalar_add(ee1, ee, 1.0)
            rr = work.tile([R, EA], F32, tag="rr")
            nc.vector.reciprocal_approx_fast(rr, ee1)
            er_ = work.tile([R, Wd], F32, tag="er_")
            nc.vector.tensor_mul(er_, ee[:, 0:Wd], rr[:, 0:Wd])      # sigmoid
            eAm = work.tile([R, Wd], F32, tag="eAm")
            nc.vector.tensor_scalar_add(eAm, ee[:, Wd:EA], -1.0)
            ad_ = work.tile([R, Wd], F32, tag="ad_")
            nc.vector.tensor_mul(ad_, eAm, rr[:, Wd:EA])             # tanh

            # sharpening + normalize
            lnw = work.tile([64, S], F32, tag="lnw")
            nc.scalar.activation(lnw, sw, AF.Ln)
            sp = work.tile([64, S], F32, tag="sp")
            sps = work.tile([64, 1], F32, tag="sps")
            nc.scalar.activation(sp, lnw, AF.Exp, scale=gam1, accum_out=sps)
            nc.vector.tensor_scalar_add(sps, sps, 1e-6)
            spr = work.tile([64, 1], F32, tag="spr")
            nc.vector.reciprocal_approx_fast(spr, sps)
            st16 = work.tile([64, S], F16, tag="st16")
            nc.vector.tensor_scalar_mul(st16, sp, spr)              # stacked read_w/write_w

            # out h-part early; accumulation group stays open for the rv part
            psOut = ph.tile([R, OUT], F32, tag="psg")
            for k in range(HK):
                nc.tensor.matmul(psOut, hT[:, k, :], woh_v[:, k, :],
                                 start=(k == 0), stop=False, skip_group_check=True)

            # transpose stacked weights; head product in transposed space;
            # t1 = read_w @ [mem_bias|1] ; t2|srw = (read_w*write_w) @ [mem_bias|1]
            stT = work.tile([128, 4, 64], F16, tag="stT")
            rwT = work.tile([128, 4, R], F16, tag="rwT")
            psT1 = pbig.tile([R, Wd + 1], F32, tag="pgA")
            psT2 = pbig.tile([R, Wd + 1], F32, tag="pgB")
            for j in range(4):
                tp = pt.tile([128, 64], F16, tag="tp")
                nc.tensor.transpose(tp, st16[:, 128 * j : 128 * (j + 1)], ident)
                nc.vector.tensor_copy(stT[:, j, :], tp)
                nc.vector.tensor_mul(rwT[:, j, :], stT[:, j, 0:R], stT[:, j, R:64])
                nc.tensor.matmul(psT1, stT[:, j, 0:R], mbe_v[:, j, :], start=(j == 0), stop=(j == 3))
                nc.tensor.matmul(psT2, rwT[:, j, :], mbe_v[:, j, :], start=(j == 0), stop=(j == 3))


            # rv = t1 - erase*t2 + add*srw
            x1 = work.tile([R, Wd], F32, tag="x1")
            nc.vector.tensor_mul(x1, er_, psT2[:, 0:Wd])
            x2 = work.tile([R, Wd], F32, tag="x2")
            nc.vector.tensor_sub(x2, psT1[:, 0:Wd], x1)
            x3 = work.tile([R, Wd], F32, tag="x3")
            nc.vector.tensor_scalar_mul(x3, ad_, psT2[:, Wd : Wd + 1])
            rv16 = work.tile([R, Wd], F16, tag="rv16")
            nc.vector.tensor_add(rv16, x2, x3)

            # rv.T (+ones row) closes the out accumulation
            rvT = singles.tile([128, 3, R], F16, tag="rvT")
            nc.vector.memset(rvT[0:1, 2, :], 1.0)
            for j in range(2):
                tp = pt.tile([128, R], F16, tag="tp")
                nc.tensor.transpose(tp, rv16[:, 128 * j : 128 * (j + 1)], ident[0:R, 0:R])
                nc.vector.tensor_copy(rvT[:, j, :], tp)
            nc.tensor.matmul(psOut, rvT[:, 0, :], wrv_v[:, 0, :], start=False, stop=False,
                             skip_group_check=True)
            nc.tensor.matmul(psOut, rvT[:, 1, :], wrv_v[:, 1, :], start=False, stop=False,
                             skip_group_check=True)
            nc.tensor.matmul(psOut, rvT[0:1, 2, :], brow_o, start=False, stop=True,
                             skip_group_check=True)

            yout = work.tile([R, OUT], F32, tag="yout")
            nc.vector.tensor_copy(yout, psOut)
            nc.sync.dma_start(out=y[:], in_=yout[:])

    nc.finalize()
    return nc


def _kp(a, kc):
    """[kc*128, c] -> [128, kc*c] partition-major packed."""
    c = a.shape[1]
    return a.reshape(kc, 128, c).transpose(1, 0, 2).reshape(128, kc * c)


def _prep(inputs):
    f16 = np.float16
    x = np.asarray(inputs["x"], np.float32)
    W_ih = np.asarray(inputs["W_ih"], np.float32)
    b_ih = np.asarray(inputs["b_ih"], np.float32)
    b_hh = np.asarray(inputs["b_hh"], np.float32)
    W_read = np.asarray(inputs["W_read"], np.float32)
    b_read = np.asarray(inputs["b_read"], np.float32)
    W_write = np.asarray(inputs["W_write"], np.float32)
    b_write = np.asarray(inputs["b_write"], np.float32)
    W_out = np.asarray(inputs["W_out"], np.float32)
    b_out = np.asarray(inputs["b_out"], np.float32)
    mem_bias = np.asarray(inputs["mem_bias"], np.float32)

    i0, g0, o0 = 0, 2 * H, 3 * H
    wih = np.zeros((8, 128, 1536), f16)
    wbias = np.zeros((2, 3072), f16)
    for m in range(8):
        sl = [slice(b0 + 128 * m, b0 + 128 * m + 128) for b0 in (i0, g0, o0)]
        blk = np.concatenate([W_ih[s, :IN] for s in sl], axis=0)  # [384, 512]
        wih[m] = _kp(blk.T.astype(f16), 4)
        for j, s in enumerate(sl):
            wbias[0, (m * 3 + j) * 128 : (m * 3 + j + 1) * 128] = b_ih[s].astype(f16)
            wbias[1, (m * 3 + j) * 128 : (m * 3 + j + 1) * 128] = b_hh[s].astype(f16)

    wrtwwa = np.concatenate([
        _kp(W_read.T.astype(f16), 8),               # [128, 2096]
        _kp(W_write[:P_READ, :].T.astype(f16), 8),  # [128, 2096]
    ], axis=1)
    wea_d = _kp(W_write[P_READ:, :].T.astype(f16), 8)   # [128, 4096]
    mbt_d = _kp(mem_bias.T.astype(f16), 2)              # [128, 1024]
    mbe = np.concatenate([mem_bias, np.ones((S, 1), np.float32)], axis=1).astype(f16)
    mbe_d = _kp(mbe, 4)                                  # [128, 1028]

    cst_a = np.zeros((128, 97), f16)
    cst_a[0:64, 0:64] = np.eye(64, dtype=f16)
    cst_a[:, 64] = 1.0
    cst_a[0:2, 65:97] = 1.0
    cst_b = np.zeros((1, 1676), f16)
    cst_b[0, 0:128] = 1.0
    cst_b[0, 128 : 128 + P_READ] = b_read.astype(f16)
    cst_b[0, 390 : 390 + P_READ] = b_write[:P_READ].astype(f16)
    cst_b[0, 652 : 652 + EA] = b_write[P_READ:].astype(f16)
    cst_b[0, 1164 : 1164 + OUT] = b_out.astype(f16)

    tail3 = np.concatenate([
        _kp(W_out[:, :H].T.astype(f16), 8),         # [128, 4096]
        _kp(W_out[:, H:].T.astype(f16), 2),         # [128, 1024]
    ], axis=1)

    shared = dict(wih=wih, wbias=wbias, cst_a=cst_a, cst_b=cst_b, wrtwwa=wrtwwa,
                  mbt_d=mbt_d, wea_d=wea_d, mbe_d=mbe_d, tail3=tail3)
    in_maps = []
    for c in range(NC_):
        m = dict(shared)
        m["xT"] = _kp(np.ascontiguousarray(x[c * R : (c + 1) * R].T).astype(f16), 4)
        in_maps.append(m)
    return in_maps


def kernel(**inputs) -> np.ndarray:
    if "nc" not in _cache:
        _cache["nc"] = _build()
    nc = _cache["nc"]
    in_maps = _prep(inputs)
    res = run_bass_kernel_spmd(nc, in_maps, core_ids=list(range(NC_)))
    return np.concatenate([res.results[c]["y"] for c in range(NC_)], axis=0)


# revision 23
# speedup vs baseline: 1.0800x; 1.0109x over previous
"""Trainium2 Bass kernel for nn_NeuralTuringMachine (single NTM cell forward).

Math (algebraically reduced from the reference):
  - initial state: h=c=0, rv=0, memory = broadcast(mem_bias), w0 = one-hot(0)
  - gates = x @ W_ih[:, :IN].T + b_ih + b_hh        (rv=0 -> last Wd cols unused;
    f-gate unused since c=0)
  - h = sigmoid(o) * tanh(sigmoid(i) * tanh(g))
  - rp = h @ W_read.T + b_read ; wp = h @ W_write.T + b_write
  - read_w / write_w: content softmax + interpolation gate + circular shift +
    sharpening (memory rows equal mem_bias pre-write, so content addressing is
    a plain matmul against mem_bias)
  - rv = read_w @ mem_bias - erase * ((read_w*write_w) @ mem_bias)
         + add * sum_s(read_w*write_w)        (memory tensor never materialized)
  - out = [h | rv] @ W_out.T + b_out

Sharding: data-parallel over batch (8 cores x 32 rows); weights replicated
per-core in fp16, packed host-side into partition-major contiguous blocks so
each DMA moves multi-KB runs per partition. The LSTM phase runs transposed
(gates.T) so h.T lands directly in the lhsT layout the tail matmuls need and
gate biases are per-partition ACT biases. Two-head addressing runs stacked on
64 partitions (read head rows 0:32, write head rows 32:64). The tail uses only
Exp/Ln activations (sigmoid/tanh rebuilt from exp on DVE) so the ACT engine
loads exactly two function tables, both hidden under DMA waits.
"""

import sys

for _p in ("/opt/trn_rl_repo",):
    if _p not in sys.path:
        sys.path.insert(0, _p)

import numpy as np

import concourse.bass as bass
import concourse.mybir as mybir
from concourse import bacc
from concourse.bass_utils import run_bass_kernel_spmd
from concourse.tile import TileContext, add_dep_helper

# The act-table placement pass picks the FIRST act_func_set containing each
# function; exp and ln then resolve to different tables and every exp<->ln
# alternation costs a ~1.3us table load. Empty those two sets (preserving
# list positions, which are the hardware set ids) so both resolve to
# natural_log_exp_and_others.
_orig_get_act_tables = bacc.get_activation_tables


def _patched_get_act_tables(arch):
    t = dict(_orig_get_act_tables(arch))
    for name in ("exp_and_others", "natural_log"):
        if name in t:
            t[name] = set()
    return t


bacc.get_activation_tables = _patched_get_act_tables

F16 = mybir.dt.float16
F32 = mybir.dt.float32
AF = mybir.ActivationFunctionType
MUL = mybir.AluOpType.mult
ADD = mybir.AluOpType.add

B, IN, H, OUT = 256, 512, 1024, 512
S, Wd, SH = 512, 256, 3
P_READ = Wd + 2 + SH + 1          # 262
NC_ = 8                           # cores
R = B // NC_                      # 32 batch rows per core
HK = H // 128                     # 8 h-chunks
EA = 2 * Wd                       # erase|add width 512

# tail2 packed layout (columns)
T2_MBT = 0                        # [128, 2, 512] fp16  (mem_bias.T, w-major chunks)
T2_MBE = T2_MBT + 2 * S           # [128, 4, 257]       (mem_bias | ones col, s-major)
T2_BR = T2_MBE + 4 * (Wd + 1)     # row0: b_read [262]
T2_BWA = T2_BR + P_READ           # row0: b_write[:262]
T2_BEA = T2_BWA + P_READ          # row0: b_write[262:]
T2_BO = T2_BEA + EA               # row0: b_out [512]
T2_ID = T2_BO + OUT               # rows 0:64: identity 64x64
T2_ONE128 = T2_ID + 64            # all rows: 1.0 (1 col)
T2_ONE64 = T2_ONE128 + 1          # row0: ones [64]
T2_COLS = T2_ONE64 + 64

_cache = {}


def _build():
    nc = bacc.Bacc(trn_type="TRN2")

    xT = nc.dram_tensor("xT", [128, 128], F16, kind="ExternalInput")
    wih = nc.dram_tensor("wih", [8, 128, 1536], F16, kind="ExternalInput")
    wbias = nc.dram_tensor("wbias", [2, 3072], F16, kind="ExternalInput")
    cst_a = nc.dram_tensor("cst_a", [128, 97], F16, kind="ExternalInput")
    cst_b = nc.dram_tensor("cst_b", [1, 1676], F16, kind="ExternalInput")
    wrtwwa = nc.dram_tensor("wrtwwa", [128, 4192], F16, kind="ExternalInput")
    mbt_d = nc.dram_tensor("mbt_d", [128, 1024], F16, kind="ExternalInput")
    tail3 = nc.dram_tensor("tail3", [128, 5120], F16, kind="ExternalInput")
    wea_d = nc.dram_tensor("wea_d", [128, 4096], F16, kind="ExternalInput")
    mbe_d = nc.dram_tensor("mbe_d", [128, 1028], F16, kind="ExternalInput")
    y = nc.dram_tensor("y", [R, OUT], F32, kind="ExternalOutput")

    with TileContext(nc) as tc:
        import contextlib

        with contextlib.ExitStack() as ctx:
            singles = ctx.enter_context(tc.tile_pool(name="singles", bufs=1))
            wm_pool = ctx.enter_context(tc.tile_pool(name="wm", bufs=8))
            work = ctx.enter_context(tc.tile_pool(name="work", bufs=2))
            ph = ctx.enter_context(tc.tile_pool(name="ph", bufs=2, space="PSUM"))
            pt = ctx.enter_context(tc.tile_pool(name="pt", bufs=2, space="PSUM"))
            pbig = ctx.enter_context(tc.tile_pool(name="pbig", bufs=1, space="PSUM"))

            # ---------- DMAs (issue order ~ priority) ----------
            t_xt = singles.tile([128, 128], F16, tag="xt")
            _dmas = []

            def _odma(out, in_):
                d = nc.sync.dma_start(out=out, in_=in_)
                if _dmas:
                    add_dep_helper(d.ins, _dmas[-1].ins, sync=False,
                                   reason="input dma issue order")
                _dmas.append(d)
                return d

            t_mbt = singles.tile([128, 1024], F16, tag="mbt")
            t_ca = singles.tile([128, 97], F16, tag="ca")
            t_cb = singles.tile([1, 1676], F16, tag="cb")
            _odma(t_ca[:], cst_a[:])
            _odma(t_mbt[:], mbt_d[:])
            _odma(t_xt[:], xT[:])
            _odma(t_cb[:], cst_b[:])
            t_wb = singles.tile([2, 3072], F16, tag="wb")
            _odma(t_wb[:], wbias[:])
            t_wm = []
            for m in range(8):
                t = wm_pool.tile([128, 1536], F16, tag="wm")
                _odma(t[:], wih[m])
                t_wm.append(t)
            t_rw = singles.tile([128, 4192], F16, tag="rw")
            _odma(t_rw[:], wrtwwa[:])
            t_we = singles.tile([128, 4096], F16, tag="we")
            _odma(t_we[:], wea_d[:])
            t_mbe = singles.tile([128, 1028], F16, tag="mbe")
            _odma(t_mbe[:], mbe_d[:])
            t_t3 = singles.tile([128, 5120], F16, tag="t3")
            _odma(t_t3[:], tail3[:])

            # views into packed tiles
            xt = t_xt[:].rearrange("p (k r) -> p k r", k=4)
            wrt_v = t_rw[:, 0:2096].rearrange("p (k c) -> p k c", k=8)
            wwa_v = t_rw[:, 2096:4192].rearrange("p (k c) -> p k c", k=8)
            wea_v = t_we[:].rearrange("p (k c) -> p k c", k=8)
            mbt_v = t_mbt[:].rearrange("p (k c) -> p k c", k=2)
            mbe_v = t_mbe[:].rearrange("p (k c) -> p k c", k=4)
            ident = t_ca[0:64, 0:64]
            ones128 = t_ca[:, 64:65]
            ones2 = t_ca[0:2, 65:97]
            onesrow = t_cb[0:1, 0:128]
            ones64 = t_cb[0:1, 0:64]
            brow_r = t_cb[0:1, 128 : 128 + P_READ]
            brow_wa = t_cb[0:1, 390 : 390 + P_READ]
            brow_ea = t_cb[0:1, 652 : 652 + EA]
            brow_o = t_cb[0:1, 1164 : 1164 + OUT]
            woh_v = t_t3[:, 0:4096].rearrange("p (k c) -> p k c", k=8)
            wrv_v = t_t3[:, 4096:5120].rearrange("p (k c) -> p k c", k=2)

            # mn^2 = column sums of mbt^2 — runs at kernel start, before the
            # first sigmoid, so its Ln/Exp use the initial exp table window
            mbt2 = singles.tile([128, 2, S], F16, tag="mbt2")
            nc.gpsimd.tensor_mul(mbt2[:], mbt_v[:, :, :], mbt_v[:, :, :])
            ps_mn = pbig.tile([1, S], F32, tag="pgB")
            for k in range(2):
                nc.tensor.matmul(ps_mn, ones128, mbt2[:, k, :], start=(k == 0), stop=(k == 1))
            mn_l = singles.tile([1, S], F32, tag="mn_l")
            nc.scalar.activation(mn_l, ps_mn, AF.Ln)
            nc.vector.tensor_scalar_mul(mn_l, mn_l, -0.5)
            ivmn = singles.tile([1, S], F16, tag="ivmn")
            nc.scalar.activation(ivmn, mn_l, AF.Exp)

            # ---------- phase 1: gates.T -> h.T ----------
            hT = singles.tile([128, HK + 1, R], F16, tag="hT")
            nc.vector.memset(hT[0:1, HK, :], 1.0)  # ones row (bias matmuls)

            for mp in range(4):
                m0 = 2 * mp
                psg = ph.tile([128, 2, 3 * R], F32, tag="psg")
                for mi in range(2):
                    m = m0 + mi
                    wm_v = t_wm[m][:].rearrange("p (k c) -> p k c", k=4)
                    for j in range(3):
                        for k in range(4):
                            nc.tensor.matmul(
                                psg[:, mi, j * R : (j + 1) * R],
                                wm_v[:, k, 128 * j : 128 * j + 128],
                                xt[:, k, :],
                                start=(k == 0),
                                stop=False,
                            )
                        # bias rows (b_ih, b_hh) via a K=2 rank-update
                        nc.tensor.matmul(
                            psg[:, mi, j * R : (j + 1) * R],
                            t_wb[:, (m * 3 + j) * 128 : (m * 3 + j) * 128 + 128],
                            ones2,
                            start=False,
                            stop=True,
                        )
                # [128, 2, R] activations: both m of the pair in one ACT op
                si = work.tile([128, 2, R], F32, tag="si")
                tg = work.tile([128, 2, R], F32, tag="tg")
                nc.scalar.activation(si, psg[:, :, 0:R], AF.Sigmoid)
                nc.scalar.activation(tg, psg[:, :, R : 2 * R], AF.Tanh)
                cc = work.tile([128, 2, R], F32, tag="cc")
                nc.vector.tensor_mul(cc, si, tg)
                tc_ = work.tile([128, 2, R], F32, tag="tc_")
                nc.scalar.activation(tc_, cc, AF.Tanh)
                so = work.tile([128, 2, R], F32, tag="so")
                nc.scalar.activation(so, psg[:, :, 2 * R : 3 * R], AF.Sigmoid)
                nc.vector.tensor_mul(hT[:, m0 : m0 + 2, :], so, tc_)

            # ---------- rp/wp matmuls (read head rows 0:32, write head rows 32:64) ----------
            psA1 = pbig.tile([R, P_READ], F32, tag="pgA")
            psA2 = pbig.tile([64, P_READ], F32, tag="pgA2")
            for k in range(HK + 1):
                lA = hT[:, k, :] if k < HK else hT[0:1, HK, :]
                rA = wrt_v[:, k, :] if k < HK else brow_r
                rB = wwa_v[:, k, :] if k < HK else brow_wa
                nc.tensor.matmul(psA1, lA, rA, start=(k == 0), stop=(k == HK))
                nc.tensor.matmul(psA2[R:64, :], lA, rB, start=(k == 0), stop=(k == HK))

            # broadcast inv_mn over all partitions; pre-normalize mbt columns
            psB = pbig.tile([128, S], F32, tag="pgB")
            nc.tensor.matmul(psB, onesrow, ivmn[:], start=True, stop=True)
            ivb16 = singles.tile([128, S], F16, tag="ivb16")
            nc.scalar.activation(ivb16, psB, AF.Identity)
            mbtN = singles.tile([128, 2, S], F16, tag="mbtN")
            nc.gpsimd.tensor_mul(mbtN[:, 0, :], mbt_v[:, 0, :], ivb16)
            nc.gpsimd.tensor_mul(mbtN[:, 1, :], mbt_v[:, 1, :], ivb16)


            # ---------- addressing (all-exp tail) ----------
            scal = work.tile([64, 6], F32, tag="scal")
            nc.vector.tensor_copy(scal[0:R, :], psA1[:, Wd : Wd + 6])
            nc.vector.tensor_copy(scal[R:64, :], psA2[R:64, Wd : Wd + 6])
            kst = work.tile([64, Wd], F16, tag="kst")
            nc.vector.tensor_copy(kst[0:R, :], psA1[:, 0:Wd])
            nc.scalar.activation(kst[R:64, :], psA2[R:64, 0:Wd], AF.Identity)


            # per-head scalars, all from one Exp over scal
            es = work.tile([64, 6], F32, tag="es")
            nc.scalar.activation(es, scal, AF.Exp)
            lnin = work.tile([64, 2], F32, tag="lnin")
            nc.vector.tensor_scalar_add(lnin, es[:, 0:6:5], 1.0)
            bg_l = work.tile([64, 2], F32, tag="bg_l")
            nc.scalar.activation(bg_l, lnin, AF.Ln)            # softplus(beta), softplus(gamma)
            gam1 = work.tile([64, 1], F32, tag="gam1")
            nc.vector.tensor_scalar_add(gam1, bg_l[:, 1:2], 1.0)
            rec_in = work.tile([64, 2], F32, tag="rec_in")
            nc.vector.tensor_scalar_add(rec_in[:, 0:1], es[:, 1:2], 1.0)
            nc.vector.reduce_sum(rec_in[:, 1:2], es[:, 2:5], axis=mybir.AxisListType.X)
            rec = work.tile([64, 2], F32, tag="rec")
            nc.vector.reciprocal(rec, rec_in)
            og = rec[:, 0:1]                                   # og = 1 - gate
            gate = work.tile([64, 1], F32, tag="gate")
            nc.vector.tensor_mul(gate, es[:, 1:2], og)         # sigmoid(gate)
            shn = work.tile([64, SH], F32, tag="shn")
            nc.vector.tensor_scalar_mul(shn, es[:, 2:5], rec[:, 1:2])

            # beta / ||key||
            kn2 = work.tile([64, Wd], F32, tag="kn2")
            nc.vector.tensor_mul(kn2, kst, kst)
            kn2s = work.tile([64, 1], F32, tag="kn2s")
            nc.vector.reduce_sum(kn2s, kn2, axis=mybir.AxisListType.X)
            kn_l = work.tile([64, 1], F32, tag="kn_l")
            nc.scalar.activation(kn_l, kn2s, AF.Ln)
            nc.vector.tensor_scalar_mul(kn_l, kn_l, -0.5)
            ikn = work.tile([64, 1], F32, tag="ikn")
            nc.scalar.activation(ikn, kn_l, AF.Exp)
            bikn = work.tile([64, 1], F32, tag="bikn")
            nc.vector.tensor_mul(bikn, bg_l[:, 0:1], ikn)

            # key.T then content scores
            kT = work.tile([128, 2, 64], F16, tag="kT")
            for j in range(2):
                tp = pt.tile([128, 64], F16, tag="tp")
                nc.tensor.transpose(tp, kst[:, 128 * j : 128 * (j + 1)], ident)
                nc.vector.tensor_copy(kT[:, j, :], tp)
            psN = pbig.tile([64, S], F32, tag="pgB")
            for j in range(2):
                nc.tensor.matmul(psN, kT[:, j, :], mbtN[:, j, :], start=(j == 0), stop=(j == 1))

            # erase|add pre-activations (consumed late, scheduled off-chain)
            psC = pbig.tile([R, EA], F32, tag="pgC")
            for k in range(HK + 1):
                lA = hT[:, k, :] if k < HK else hT[0:1, HK, :]
                rC = wea_v[:, k, :] if k < HK else brow_ea
                nc.tensor.matmul(psC, lA, rC, start=(k == 0), stop=(k == HK))

            ex = work.tile([64, S], F32, tag="ex")
            exs = work.tile([64, 1], F32, tag="exs")
            nc.scalar.activation(ex, psN, AF.Exp, scale=bikn, accum_out=exs)
            exr = work.tile([64, 1], F32, tag="exr")
            nc.vector.reciprocal_approx_fast(exr, exs)
            gcw = work.tile([64, 1], F32, tag="gcw")
            nc.vector.tensor_mul(gcw, gate, exr)
            gw = work.tile([64, S], F32, tag="gw")
            nc.vector.tensor_scalar_mul(gw, ex, gcw)              # gate * softmax
            nc.vector.tensor_add(gw[:, 0:1], gw[:, 0:1], og)      # + (1-gate) * w0

            # erase/add exps (ACT-side; DVE part comes after the shift)
            ee = work.tile([R, EA], F32, tag="ee")
            nc.scalar.activation(ee[:, 0:Wd], psC[:, 0:Wd], AF.Exp)
            nc.scalar.activation(ee[:, Wd:EA], psC[:, Wd:EA], AF.Exp, scale=2.0)

            # circular shift (mul+add pairs fused via scalar_tensor_tensor)
            sw = work.tile([64, S], F32, tag="sw")
            nc.vector.tensor_scalar_mul(sw, gw, shn[:, 1:2])
            nc.vector.scalar_tensor_tensor(
                sw[:, 1:S], gw[:, 0 : S - 1], shn[:, 0:1], sw[:, 1:S], MUL, ADD)
            nc.vector.scalar_tensor_tensor(
                sw[:, 0:1], gw[:, S - 1 : S], shn[:, 0:1], sw[:, 0:1], MUL, ADD)
            nc.vector.scalar_tensor_tensor(
                sw[:, 0 : S - 1], gw[:, 1:S], shn[:, 2:3], sw[:, 0 : S - 1], MUL, ADD)
            nc.vector.scalar_tensor_tensor(
                sw[:, S - 1 : S], gw[:, 0:1], shn[:, 2:3], sw[:, S - 1 : S], MUL, ADD)


            # sharpening + normalize
            lnw = work.tile([64, S], F32, tag="lnw")
            nc.scalar.activation(lnw, sw, AF.Ln)
            sp = work.tile([64, S], F32, tag="sp")
            sps = work.tile([64, 1], F32, tag="sps")
            nc.scalar.activation(sp, lnw, AF.Exp, scale=gam1, accum_out=sps)
            nc.vector.tensor_scalar_add(sps, sps, 1e-6)
            spr = work.tile([64, 1], F32, tag="spr")
            nc.vector.reciprocal_approx_fast(spr, sps)
            st16 = work.tile([64, S], F16, tag="st16")
            nc.vector.tensor_scalar_mul(st16, sp, spr)              # stacked read_w/write_w

            # out h-part early; accumulation group stays open for the rv part
            psOut = ph.tile([R, OUT], F32, tag="psg")
            for k in range(HK):
                nc.tensor.matmul(psOut, hT[:, k, :], woh_v[:, k, :],
                                 start=(k == 0), stop=False, skip_group_check=True)

            # erase/add DVE part (fills DVE while ACT does Ln/Exp sharpening)
            ee1 = work.tile([R, EA], F32, tag="ee1")
            nc.vector.tensor_scalar_add(ee1, ee, 1.0)
            rr = work.tile([R, EA], F32, tag="rr")
            nc.vector.reciprocal_approx_fast(rr, ee1)
            er_ = work.tile([R, Wd], F32, tag="er_")
            nc.vector.tensor_mul(er_, ee[:, 0:Wd], rr[:, 0:Wd])      # sigmoid
            eAm = work.tile([R, Wd], F32, tag="eAm")
            nc.vector.tensor_scalar_add(eAm, ee[:, Wd:EA], -1.0)
            ad_ = work.tile([R, Wd], F32, tag="ad_")
            nc.vector.tensor_mul(ad_, eAm, rr[:, Wd:EA])             # tanh

            # transpose stacked weights; head product in transposed space;
            # t1 = read_w @ [mem_bias|1] ; t2|srw = (read_w*write_w) @ [mem_bias|1]
            stT = work.tile([128, 4, 64], F16, tag="stT")
            rwT = work.tile([128, 4, R], F16, tag="rwT")
            psT1 = pbig.tile([R, Wd + 1], F32, tag="pgA")
            psT2 = pbig.tile([R, Wd + 1], F32, tag="pgB")
            for j in range(4):
                tp = pt.tile([128, 64], F16, tag="tp")
                nc.tensor.transpose(tp, st16[:, 128 * j : 128 * (j + 1)], ident)
                nc.vector.tensor_copy(stT[:, j, :], tp)
                nc.vector.tensor_mul(rwT[:, j, :], stT[:, j, 0:R], stT[:, j, R:64])
                nc.tensor.matmul(psT1, stT[:, j, 0:R], mbe_v[:, j, :], start=(j == 0), stop=(j == 3))
                nc.tensor.matmul(psT2, rwT[:, j, :], mbe_v[:, j, :], start=(j == 0), stop=(j == 3))


            # rv = t1 - erase*t2 + add*srw
            x1 = work.tile([R, Wd], F32, tag="x1")
            nc.vector.tensor_mul(x1, er_, psT2[:, 0:Wd])
            x2 = work.tile([R, Wd], F32, tag="x2")
            nc.vector.tensor_sub(x2, psT1[:, 0:Wd], x1)
            x3 = work.tile([R, Wd], F32, tag="x3")
            nc.vector.tensor_scalar_mul(x3, ad_, psT2[:, Wd : Wd + 1])
            rv16 = work.tile([R, Wd], F16, tag="rv16")
            nc.vector.tensor_add(rv16, x2, x3)

            # rv.T (+ones row) closes the out accumulation
            rvT = singles.tile([128, 3, R], F16, tag="rvT")
            nc.vector.memset(rvT[0:1, 2, :], 1.0)
            for j in range(2):
                tp = pt.tile([128, R], F16, tag="tp")
                nc.tensor.transpose(tp, rv16[:, 128 * j : 128 * (j + 1)], ident[0:R, 0:R])
                nc.vector.tensor_copy(rvT[:, j, :], tp)
            nc.tensor.matmul(psOut, rvT[:, 0, :], wrv_v[:, 0, :], start=False, stop=False,
                             skip_group_check=True)
            nc.tensor.matmul(psOut, rvT[:, 1, :], wrv_v[:, 1, :], start=False, stop=False,
                             skip_group_check=True)
            nc.tensor.matmul(psOut, rvT[0:1, 2, :], brow_o, start=False, stop=True,
                             skip_group_check=True)

            yout = work.tile([R, OUT], F32, tag="yout")
            nc.vector.tensor_copy(yout, psOut)
            nc.sync.dma_start(out=y[:], in_=yout[:])

    nc.finalize()
    return nc


def _kp(a, kc):
    """[kc*128, c] -> [128, kc*c] partition-major packed."""
    c = a.shape[1]
    return a.reshape(kc, 128, c).transpose(1, 0, 2).reshape(128, kc * c)


def _prep(inputs):
    f16 = np.float16
    x = np.asarray(inputs["x"], np.float32)
    W_ih = np.asarray(inputs["W_ih"], np.float32)
    b_ih = np.asarray(inputs["b_ih"], np.float32)
    b_hh = np.asarray(inputs["b_hh"], np.float32)
    W_read = np.asarray(inputs["W_read"], np.float32)
    b_read = np.asarray(inputs["b_read"], np.float32)
    W_write = np.asarray(inputs["W_write"], np.float32)
    b_write = np.asarray(inputs["b_write"], np.float32)
    W_out = np.asarray(inputs["W_out"], np.float32)
    b_out = np.asarray(inputs["b_out"], np.float32)
    mem_bias = np.asarray(inputs["mem_bias"], np.float32)

    i0, g0, o0 = 0, 2 * H, 3 * H
    wih = np.zeros((8, 128, 1536), f16)
    wbias = np.zeros((2, 3072), f16)
    for m in range(8):
        sl = [slice(b0 + 128 * m, b0 + 128 * m + 128) for b0 in (i0, g0, o0)]
        blk = np.concatenate([W_ih[s, :IN] for s in sl], axis=0)  # [384, 512]
        wih[m] = _kp(blk.T.astype(f16), 4)
        for j, s in enumerate(sl):
            wbias[0, (m * 3 + j) * 128 : (m * 3 + j + 1) * 128] = b_ih[s].astype(f16)
            wbias[1, (m * 3 + j) * 128 : (m * 3 + j + 1) * 128] = b_hh[s].astype(f16)

    wrtwwa = np.concatenate([
        _kp(W_read.T.astype(f16), 8),               # [128, 2096]
        _kp(W_write[:P_READ, :].T.astype(f16), 8),  # [128, 2096]
    ], axis=1)
    wea_d = _kp(W_write[P_READ:, :].T.astype(f16), 8)   # [128, 4096]
    mbt_d = _kp(mem_bias.T.astype(f16), 2)              # [128, 1024]
    mbe = np.concatenate([mem_bias, np.ones((S, 1), np.float32)], axis=1).astype(f16)
    mbe_d = _kp(mbe, 4)                                  # [128, 1028]

    cst_a = np.zeros((128, 97), f16)
    cst_a[0:64, 0:64] = np.eye(64, dtype=f16)
    cst_a[:, 64] = 1.0
    cst_a[0:2, 65:97] = 1.0
    cst_b = np.zeros((1, 1676), f16)
    cst_b[0, 0:128] = 1.0
    cst_b[0, 128 : 128 + P_READ] = b_read.astype(f16)
    cst_b[0, 390 : 390 + P_READ] = b_write[:P_READ].astype(f16)
    cst_b[0, 652 : 652 + EA] = b_write[P_READ:].astype(f16)
    cst_b[0, 1164 : 1164 + OUT] = b_out.astype(f16)

    tail3 = np.concatenate([
        _kp(W_out[:, :H].T.astype(f16), 8),         # [128, 4096]
        _kp(W_out[:, H:].T.astype(f16), 2),         # [128, 1024]
    ], axis=1)

    shared = dict(wih=wih, wbias=wbias, cst_a=cst_a, cst_b=cst_b, wrtwwa=wrtwwa,
                  mbt_d=mbt_d, wea_d=wea_d, mbe_d=mbe_d, tail3=tail3)
    in_maps = []
    for c in range(NC_):
        m = dict(shared)
        m["xT"] = _kp(np.ascontiguousarray(x[c * R : (c + 1) * R].T).astype(f16), 4)
        in_maps.append(m)
    return in_maps


def kernel(**inputs) -> np.ndarray:
    if "nc" not in _cache:
        _cache["nc"] = _build()
    nc = _cache["nc"]
    in_maps = _prep(inputs)
    res = run_bass_kernel_spmd(nc, in_maps, core_ids=list(range(NC_)))
    return np.concatenate([res.results[c]["y"] for c in range(NC_)], axis=0)
